# revision 2
# baseline (speedup 1.0000x reference)
"""Bass/Trainium2 kernel for nn_BitwiseTasNetRepeat.

Strategy (v4: single-pass depthwise + STT fusion + 4-engine balance)
--------------------------------------------------------------------
Each block collapses to threshold arithmetic on binary values:

    S1 = (R >= t1)                      {0,1} fp8   (GpSimd tensor_scalar)
    p1 = W1s @ S1                       (TensorE fp8 DR, K=256; rowsum
                                         of W1s folded into tau2')
    S2n = sign(tau2' - p1)              = -s, +-1 fp8 (ACT, scale=-1)
    pd = a0*s(-d) + a2*s(+d)            (ONE TensorE DR matmul: taps at
                                         -d,+d via pair-stride 2d, diag
                                         weights -a0 / -a2 on S2n)
    S3 = ((pd - tau3) >= S2n(0))        = (q >= tau3), {0,1} fp8
                                        (DVE scalar_tensor_tensor: center
                                         tap fused into the compare)
    p2 = (2*W2*ctr) @ S3                (TensorE fp8 DR, K=512)
    R  = (p2 - corr) + R                (DVE scalar_tensor_tensor;
                                         corr = rowsum of W2*ctr)

beta-mix: mh tiles listed in _PATH as 'A' use the 3-tap TensorE dw
(2 matmuls) + ACT sign S3 (+-1 convention, W2 cols stay +-1) to move
work from DVE to ACT/TensorE.

All values are exact in fp8e4m3/fp32-PSUM.  Data-parallel over batch,
2 samples per core, 8 cores.  Stage A of step k+1 interleaves chunk-
wise with stage B/C of step k so every engine streams.
"""

import numpy as np
import ml_dtypes

_B, _CB, _H, _T = 16, 256, 512, 4096
_BLOCKS = 8
_EPS = 1e-5
_NCORES = 8
_BS = _B // _NCORES      # batches per core
_KC = _CB // 128         # 2  k-tiles of Cb
_MH = _H // 128          # 4  m-tiles of H
_PAD = 128               # halo for dilated depthwise conv (max d = 128)
_NCC = 12                # f32 const columns per block
_QC = 1024               # chunk width for dw/S3/conv2 stages

# per-mh path: 'S' = single-MM dw + DVE STT S3 ({0,1}); 'A' = 2-MM dw +
# ACT sign S3 (+-1)
_PATH = ('A', 'S', 'S', 'S')
_S1_ENGINE = 'gpsimd'    # 'gpsimd' | 'vector'

_nc_cache = {}


def _mk3(ap2d, j_step, cols):
    """3D AP [128, 2 (stride j_step), cols] over a 2D row view."""
    import bass_rust
    v = ap2d.copy()
    l = v.ap
    v.ap = bass_rust.VecI64Pair([list(l[0]), [j_step, 2], [1, cols]])
    return v


def _build_nc(bs=_BS, nblocks=_BLOCKS, T=_T):
    import concourse.mybir as mybir
    from concourse import bacc
    from concourse.tile import TileContext

    f32 = mybir.dt.float32
    fp8 = mybir.dt.float8e4
    ALU = mybir.AluOpType
    ACTF = mybir.ActivationFunctionType
    DRM = mybir.MatmulPerfMode.DoubleRow
    nq = T // _QC

    nc = bacc.Bacc("TRN2", target_bir_lowering=False, debug=False,
                   enable_asserts=False)

    x_d = nc.dram_tensor("x", [bs, _CB, T], f32, kind="ExternalInput")
    w1_d = nc.dram_tensor("w1dr", [128, nblocks * _MH * 256], fp8,
                          kind="ExternalInput")
    w2_d = nc.dram_tensor("w2dr", [128, nblocks * _KC * 2 * 256], fp8,
                          kind="ExternalInput")
    # single-pass dw weights: pair j=0 -> diag(-a0), j=1 -> diag(-a2)
    dwp_d = nc.dram_tensor("dwp", [128, nblocks * _MH * 256], fp8,
                           kind="ExternalInput")
    # ACT-path dw weights (3 taps on S2n): [diag(-a0)|diag(-1)] + diag(-a2)
    dwn01_d = nc.dram_tensor("dwn01", [128, nblocks * _MH * 256], fp8,
                             kind="ExternalInput")
    dwn2_d = nc.dram_tensor("dwn2", [128, nblocks * _MH * 128], fp8,
                            kind="ExternalInput")
    cst_d = nc.dram_tensor("cst", [128, nblocks * _NCC], f32,
                           kind="ExternalInput")
    out_d = nc.dram_tensor("out", [bs, _CB, T], f32, kind="ExternalOutput")

    with TileContext(nc) as tc:
        with (
            tc.tile_pool(name="wpool", bufs=1) as wpool,
            tc.tile_pool(name="rpool", bufs=4) as rpool,
            tc.tile_pool(name="s1pool", bufs=4) as s1pool,
            tc.tile_pool(name="s2pool", bufs=10) as s2pool,
            tc.tile_pool(name="s3pool", bufs=8) as s3pool,
            tc.tile_pool(name="psmm", bufs=2, space="PSUM") as psmm,
            tc.tile_pool(name="psdw", bufs=2, space="PSUM") as psdw,
        ):
            w1sb = wpool.tile([128, nblocks * _MH * 256], fp8)
            nc.sync.dma_start(out=w1sb[:], in_=w1_d.ap())
            w2sb = wpool.tile([128, nblocks * _KC * 2 * 256], fp8)
            nc.sync.dma_start(out=w2sb[:], in_=w2_d.ap())
            dwpsb = wpool.tile([128, nblocks * _MH * 256], fp8)
            nc.sync.dma_start(out=dwpsb[:], in_=dwp_d.ap())
            dwn01sb = wpool.tile([128, nblocks * _MH * 256], fp8)
            nc.sync.dma_start(out=dwn01sb[:], in_=dwn01_d.ap())
            dwn2sb = wpool.tile([128, nblocks * _MH * 128], fp8)
            nc.sync.dma_start(out=dwn2sb[:], in_=dwn2_d.ap())
            cst = wpool.tile([128, nblocks * _NCC], f32)
            nc.sync.dma_start(out=cst[:], in_=cst_d.ap())

            def w1t(i, mh):
                o = (i * _MH + mh) * 256
                return _mk3(w1sb[:, o:o + 256], 128, 128)

            def w2t(i, mc, pair):
                o = (i * _KC * 2 + mc * 2 + pair) * 256
                return _mk3(w2sb[:, o:o + 256], 128, 128)

            def dwpt(i, mh):
                o = (i * _MH + mh) * 256
                return _mk3(dwpsb[:, o:o + 256], 128, 128)

            def dwn01t(i, mh):
                o = (i * _MH + mh) * 256
                return _mk3(dwn01sb[:, o:o + 256], 128, 128)

            def dwn2t(i, mh):
                o = (i * _MH + mh) * 128
                return dwn2sb[:, o:o + 128]

            def cc(i, j):
                return cst[:, i * _NCC + j:i * _NCC + j + 1]

            Rb = {}
            for b in range(bs):
                Rb[b] = []
                for kc in range(_KC):
                    rt = rpool.tile([128, T], f32, tag="R",
                                    name=f"R_b{b}_{kc}")
                    nc.sync.dma_start(
                        out=rt[:], in_=x_d.ap()[b, kc * 128:(kc + 1) * 128, :])
                    Rb[b].append(rt)

            state = {}

            def emitA_alloc(b, i):
                S1 = s1pool.tile([128, _KC * T], fp8, tag="S1",
                                 name=f"S1_b{b}_i{i}")
                S2 = []
                for mh in range(_MH):
                    s2t = s2pool.tile([128, T + 2 * _PAD], fp8, tag="S2",
                                      name=f"S2_b{b}_i{i}_{mh}")
                    nc.gpsimd.memset(s2t[:, 0:_PAD], 0.0)
                    nc.gpsimd.memset(s2t[:, _PAD + T:2 * _PAD + T], 0.0)
                    S2.append(s2t)
                state[(b, i)] = (S1, S2)

            def emitA_g(b, i, g):
                R = Rb[b]
                S1, S2 = state[(b, i)]
                c0 = g * _QC
                for kc in range(_KC):
                    # S1 = (R >= t1) in {0,1}
                    if _S1_ENGINE == 'gpsimd':
                        nc.gpsimd.tensor_scalar(
                            S1[:, kc * T + c0:kc * T + c0 + _QC],
                            R[kc][:, c0:c0 + _QC],
                            cc(i, kc), 0.0, op0=ALU.is_ge, op1=ALU.subtract)
                    else:
                        nc.vector.tensor_scalar(
                            S1[:, kc * T + c0:kc * T + c0 + _QC],
                            R[kc][:, c0:c0 + _QC],
                            cc(i, kc), 0.0, op0=ALU.is_ge, op1=ALU.subtract)
                for mh in range(_MH):
                    ps = psmm.tile([128, _QC], f32, tag="mm",
                                   name=f"psA_{b}_{i}_{mh}_{g}")
                    for nn in range(2):
                        cn = c0 + nn * 512
                        rhs = _mk3(S1[:, cn:cn + 512], T, 512)
                        nc.tensor.matmul(
                            ps[:, nn * 512:(nn + 1) * 512],
                            w1t(i, mh), rhs, start=True, stop=True,
                            perf_mode=DRM)
                    # S2n = sign(tau2' - p1)  (negated sign of BN2 argument)
                    nc.scalar.activation(
                        S2[mh][:, _PAD + c0:_PAD + c0 + _QC],
                        ps[:], ACTF.Sign, bias=cc(i, 2 + mh), scale=-1.0)

            def emitBC_q(b, i, q):
                d = 2 ** i
                R = Rb[b]
                _, S2 = state[(b, i)]
                c0 = q * _QC
                S3 = [s3pool.tile([128, 2 * _QC], fp8, tag="S3",
                                  name=f"S3_b{b}_i{i}_q{q}_p{p}")
                      for p in range(2)]
                for mh in range(_MH):
                    pd = psdw.tile([128, _QC], f32, tag="dw",
                                   name=f"psD_{b}_{i}_q{q}_{mh}")
                    s3out = S3[mh // 2][:, (mh % 2) * _QC:(mh % 2 + 1) * _QC]
                    if _PATH[mh] == 'S':
                        for nn in range(2):
                            w0 = _PAD + c0 + nn * 512
                            # taps (-d, +d) in one DR pass, pair stride 2d
                            rhs = _mk3(S2[mh][:, w0 - d:w0 - d + 512],
                                       2 * d, 512)
                            nc.tensor.matmul(
                                pd[:, nn * 512:(nn + 1) * 512],
                                dwpt(i, mh), rhs, start=True, stop=True,
                                perf_mode=DRM)
                        # S3 = ((pd - tau3) >= -s_c) = (q >= tau3) in {0,1}
                        nc.vector.scalar_tensor_tensor(
                            s3out, pd[:], cc(i, 6 + mh),
                            S2[mh][:, _PAD + c0:_PAD + c0 + _QC],
                            op0=ALU.subtract, op1=ALU.is_ge)
                    else:
                        for nn in range(2):
                            w0 = _PAD + c0 + nn * 512
                            rhs01 = _mk3(S2[mh][:, w0 - d:w0 - d + 512],
                                         d, 512)
                            nc.tensor.matmul(
                                pd[:, nn * 512:(nn + 1) * 512],
                                dwn01t(i, mh), rhs01,
                                start=True, stop=False, perf_mode=DRM)
                            nc.tensor.matmul(
                                pd[:, nn * 512:(nn + 1) * 512],
                                dwn2t(i, mh),
                                S2[mh][:, w0 + d:w0 + d + 512],
                                start=False, stop=True)
                        # S3 = sign(q - tau3) in {-1, +1}
                        nc.scalar.sign(s3out, pd[:], bias=cc(i, 6 + mh))
                for mc in range(_KC):
                    ps2 = psmm.tile([128, _QC], f32, tag="mm",
                                    name=f"psC_{b}_{i}_q{q}_{mc}")
                    for nn in range(2):
                        for pair in range(2):
                            rhs = _mk3(S3[pair][:, nn * 512:nn * 512 + 512],
                                       _QC, 512)
                            nc.tensor.matmul(
                                ps2[:, nn * 512:(nn + 1) * 512],
                                w2t(i, mc, pair), rhs,
                                start=(pair == 0), stop=(pair == 1),
                                perf_mode=DRM)
                    # R = (p2 - corr) + R
                    nc.vector.scalar_tensor_tensor(
                        R[mc][:, c0:c0 + _QC], ps2[:], cc(i, 10 + mc),
                        R[mc][:, c0:c0 + _QC],
                        op0=ALU.subtract, op1=ALU.add)

            # software-pipelined emission: stage A of step k+1 interleaves
            # chunk-wise with stage B/C of step k so every engine streams
            seq = [(b, i) for i in range(nblocks) for b in range(bs)]
            emitA_alloc(*seq[0])
            for g in range(nq):
                emitA_g(*seq[0], g)
            for k in range(len(seq)):
                if k + 1 < len(seq):
                    emitA_alloc(*seq[k + 1])
                for g in range(nq):
                    emitBC_q(*seq[k], g)
                    if k + 1 < len(seq):
                        emitA_g(*seq[k + 1], g)
                state.pop(seq[k])

            for b in range(bs):
                for kc in range(_KC):
                    nc.sync.dma_start(
                        out=out_d.ap()[b, kc * 128:(kc + 1) * 128, :],
                        in_=Rb[b][kc][:])
    nc.finalize()
    return nc


def _prep(inputs, nblocks=_BLOCKS):
    """Host-side weight/threshold preprocessing (tiny tensors only)."""
    e4 = ml_dtypes.float8_e4m3

    def thr(g, bb, m, v):
        return (m - bb * np.sqrt(v + _EPS) / g).astype(np.float32)

    w1dr = np.zeros((128, nblocks * _MH * 256), np.float32)
    w2dr = np.zeros((128, nblocks * _KC * 2 * 256), np.float32)
    dwp = np.zeros((128, nblocks * _MH * 256), np.float32)
    dwn01 = np.zeros((128, nblocks * _MH * 256), np.float32)
    dwn2 = np.zeros((128, nblocks * _MH * 128), np.float32)
    cst = np.zeros((128, nblocks * _NCC), np.float32)
    ar = np.arange(128)
    for i in range(nblocks):
        t1 = thr(inputs['bn1_gamma'][i], inputs['bn1_beta'][i],
                 inputs['bn1_mean'][i], inputs['bn1_var'][i])          # [Cb]
        t2 = thr(inputs['bn2_gamma'][i], inputs['bn2_beta'][i],
                 inputs['bn2_mean'][i], inputs['bn2_var'][i])          # [H]
        t3 = thr(inputs['bn3_gamma'][i], inputs['bn3_beta'][i],
                 inputs['bn3_mean'][i], inputs['bn3_var'][i])          # [H]
        W1s = np.sign(inputs['w1'][i]).astype(np.float32)              # [H, Cb]
        W2s = np.sign(inputs['w2'][i]).astype(np.float32)              # [Cb, H]
        dws = np.sign(inputs['dw_w'][i]).astype(np.float32)            # [H, 3]
        ctr = dws[:, 1]
        a0 = dws[:, 0] * ctr
        a2 = dws[:, 2] * ctr
        tau3 = ctr * t3                                                # [H]
        # S1 is {0,1}: p1' = W1s@b1, true p1 = 2*p1' - rowsum(W1s):
        # sign(p1-t2) = sign(p1' - tau2'), tau2' = (rowsum+t2)/2
        rs1 = W1s.sum(axis=1)                                          # [H]
        tau2p = 0.5 * (rs1 + t2)
        # conv2 weights and corr: 'S'-path columns are {0,1} -> weight
        # 2*W2s*ctr and corr += W2s*ctr rowsum; 'A'-path stays +-1
        w2eff = W2s * ctr[None, :]                                     # [Cb, H]
        amp = np.array([2.0 if _PATH[mh] == 'S' else 1.0
                        for mh in range(_MH)])
        W2use = w2eff * np.repeat(amp, 128)[None, :]
        sel = np.repeat(np.array([_PATH[mh] == 'S'
                                  for mh in range(_MH)]), 128)
        corr = (w2eff * sel[None, :]).sum(axis=1)                      # [Cb]
        for mh in range(_MH):
            o = (i * _MH + mh) * 256
            for j in range(2):
                w1dr[:, o + j * 128:o + (j + 1) * 128] = \
                    W1s[mh * 128:(mh + 1) * 128, j * 128:(j + 1) * 128].T
        for mc in range(_KC):
            for pair in range(2):
                o = (i * _KC * 2 + mc * 2 + pair) * 256
                for j in range(2):
                    kh = pair * 2 + j
                    w2dr[:, o + j * 128:o + (j + 1) * 128] = \
                        W2use[mc * 128:(mc + 1) * 128,
                              kh * 128:(kh + 1) * 128].T
        for mh in range(_MH):
            sl = slice(mh * 128, (mh + 1) * 128)
            o = (i * _MH + mh) * 256
            o2 = (i * _MH + mh) * 128
            # single-pass dw on S2n (= -s): j0 diag(-a0) tap -d,
            # j1 diag(-a2) tap +d
            dwp[ar, o + ar] = -a0[sl]
            dwp[ar, o + 128 + ar] = -a2[sl]
            # ACT-path 3-tap dw on S2n: j0 diag(-a0) tap -d, j1 diag(-1)
            # tap 0, + diag(-a2) tap +d
            dwn01[ar, o + ar] = -a0[sl]
            dwn01[ar, o + 128 + ar] = -1.0
            dwn2[ar, o2 + ar] = -a2[sl]
        base = i * _NCC
        for kc in range(_KC):
            cst[:, base + kc] = t1[kc * 128:(kc + 1) * 128]
        for mh in range(_MH):
            sl = slice(mh * 128, (mh + 1) * 128)
            cst[:, base + 2 + mh] = tau2p[sl]
            # S3 scalar: 'S' path STT subtract -> tau3; 'A' path ACT sign
            # bias -> -tau3
            cst[:, base + 6 + mh] = (tau3[sl] if _PATH[mh] == 'S'
                                     else -tau3[sl])
        for mc in range(_KC):
            cst[:, base + 10 + mc] = corr[mc * 128:(mc + 1) * 128]
    return (w1dr.astype(e4), w2dr.astype(e4), dwp.astype(e4),
            dwn01.astype(e4), dwn2.astype(e4), cst)


def kernel(**inputs):
    inputs = {k: np.asarray(v) for k, v in inputs.items()}
    x = inputs['x'].astype(np.float32)
    w1dr, w2dr, dwp, dwn01, dwn2, cst = _prep(inputs)

    if 'nc' not in _nc_cache:
        _nc_cache['nc'] = _build_nc()
    nc = _nc_cache['nc']

    in_maps = []
    for c in range(_NCORES):
        in_maps.append({
            'x': np.ascontiguousarray(x[c * _BS:(c + 1) * _BS]),
            'w1dr': w1dr, 'w2dr': w2dr, 'dwp': dwp, 'dwn01': dwn01,
            'dwn2': dwn2, 'cst': cst,
        })

    from concourse.bass_utils import run_bass_kernel_spmd
    import os
    trace = bool(int(os.environ.get('KERNEL_TRACE', '0')))
    res = run_bass_kernel_spmd(nc, in_maps, core_ids=list(range(_NCORES)),
                               trace=trace)
    _nc_cache['last_result'] = res
    out = np.concatenate([r['out'] for r in res.results], axis=0)
    return out.astype(np.float32)


# revision 4
# speedup vs baseline: 3.4611x; 3.4611x over previous
"""Bass/Trainium2 kernel for nn_BitwiseTasNetRepeat.

Strategy (v5: single-pass dw + STT fusion + 4-engine balance)
-------------------------------------------------------------
Each block collapses to threshold arithmetic on binary values:

    S1 = (R >= t1)                      {0,1} fp8  (GpSimd tensor_tensor
                                         vs stride-0 broadcast of t1)
    p1 = W1s @ S1                       (TensorE fp8 DR, K=256; rowsum of
                                         W1s folded into tau2')
    S2n = sign(tau2' - p1)              = -s, +-1 fp8 (ACT, scale=-1)
    per (i, mh) 'S' slots (~2/3):
      pd = a0*s(-d) + a2*s(+d)          (ONE DR matmul: taps -d,+d via
                                         pair-stride 2d, diag -a0/-a2)
      S3 = ((pd - tau3) >= S2n(0))      = (q >= tau3) {0,1}
                                        (DVE scalar_tensor_tensor: center
                                         tap fused into the compare)
    per (i, mh) 'A' slots (~1/3, offloads DVE -> ACT/TensorE):
      pd = full 3-tap q                 (2 matmuls as in v3)
      S3 = sign(q - tau3)               +-1  (ACT sign)
    p2 = W2' @ S3                       (TensorE fp8 DR, K=512; 'S' cols
                                         2*W2*ctr, 'A' cols W2*ctr)
    R  = (p2 - corr) + R                (DVE STT; corr = integer rowsum
                                         of W2*ctr over 'S' slots -- adds
                                         bit-exactly, preserving the
                                         reference's fp32 R arithmetic)

S2 lives in a statically-allocated ring so the halo pads are memset once
at startup instead of every step.  Data-parallel over batch, 2 samples
per core, 8 cores.  Stage A of step k+1 interleaves chunk-wise with
stage B/C of step k so every engine streams.
"""

import numpy as np
import ml_dtypes

_B, _CB, _H, _T = 16, 256, 512, 4096
_BLOCKS = 8
_EPS = 1e-5
_NCORES = 8
_BS = _B // _NCORES      # batches per core
_KC = _CB // 128         # 2  k-tiles of Cb
_MH = _H // 128          # 4  m-tiles of H
_PAD = 128               # halo for dilated depthwise conv (max d = 128)
_NCC = 12                # f32 const columns per block
_QC = 1024               # chunk width for dw/S3/conv2 stages
_NS2 = 10                # S2 ring buffers

_S1_ENGINE = 'vector'    # 'gpsimd' | 'vector'  (gpsimd is_ge: walrus crash)
_BETA16 = 9              # 'A'-path slots per 16 (ACT-vs-DVE S3 balance)


def _path(i, mh):
    # 'A' = 3-tap dw + ACT sign S3; 'S' = single-MM dw + DVE STT S3
    return 'A' if ((i * _MH + mh) * _BETA16) % 16 < _BETA16 else 'S'


_nc_cache = {}


def _mk3(ap2d, j_step, cols):
    """3D AP [128, 2 (stride j_step), cols] over a 2D row view."""
    import bass_rust
    v = ap2d.copy()
    l = v.ap
    v.ap = bass_rust.VecI64Pair([list(l[0]), [j_step, 2], [1, cols]])
    return v


def _bcast(ap2d, cols):
    """[128, 1] AP -> [128, cols] broadcast AP (free stride 0)."""
    import bass_rust
    v = ap2d.copy()
    l = v.ap
    v.ap = bass_rust.VecI64Pair([list(l[0]), [0, cols]])
    return v


def _build_nc(bs=_BS, nblocks=_BLOCKS, T=_T):
    import concourse.mybir as mybir
    from concourse import bacc
    from concourse.tile import TileContext

    f32 = mybir.dt.float32
    fp8 = mybir.dt.float8e4
    ALU = mybir.AluOpType
    ACTF = mybir.ActivationFunctionType
    DRM = mybir.MatmulPerfMode.DoubleRow
    nq = T // _QC
    S2W = T + 2 * _PAD

    nc = bacc.Bacc("TRN2", target_bir_lowering=False, debug=False,
                   enable_asserts=False)

    x_d = nc.dram_tensor("x", [bs, _CB, T], f32, kind="ExternalInput")
    w1_d = nc.dram_tensor("w1dr", [128, nblocks * _MH * 256], fp8,
                          kind="ExternalInput")
    w2_d = nc.dram_tensor("w2dr", [128, nblocks * _KC * 2 * 256], fp8,
                          kind="ExternalInput")
    dwp_d = nc.dram_tensor("dwp", [128, nblocks * _MH * 256], fp8,
                           kind="ExternalInput")
    dwn01_d = nc.dram_tensor("dwn01", [128, nblocks * _MH * 256], fp8,
                             kind="ExternalInput")
    dwn2_d = nc.dram_tensor("dwn2", [128, nblocks * _MH * 128], fp8,
                            kind="ExternalInput")
    cst_d = nc.dram_tensor("cst", [128, nblocks * _NCC], f32,
                           kind="ExternalInput")
    out_d = nc.dram_tensor("out", [bs, _CB, T], f32, kind="ExternalOutput")

    with TileContext(nc) as tc:
        with (
            tc.tile_pool(name="wpool", bufs=1) as wpool,
            tc.tile_pool(name="rpool", bufs=4) as rpool,
            tc.tile_pool(name="s1pool", bufs=4) as s1pool,
            tc.tile_pool(name="s3pool", bufs=8) as s3pool,
            tc.tile_pool(name="psmm", bufs=2, space="PSUM") as psmm,
            tc.tile_pool(name="psdw", bufs=2, space="PSUM") as psdw,
        ):
            w1sb = wpool.tile([128, nblocks * _MH * 256], fp8)
            nc.sync.dma_start(out=w1sb[:], in_=w1_d.ap())
            w2sb = wpool.tile([128, nblocks * _KC * 2 * 256], fp8)
            nc.sync.dma_start(out=w2sb[:], in_=w2_d.ap())
            dwpsb = wpool.tile([128, nblocks * _MH * 256], fp8)
            nc.sync.dma_start(out=dwpsb[:], in_=dwp_d.ap())
            dwn01sb = wpool.tile([128, nblocks * _MH * 256], fp8)
            nc.sync.dma_start(out=dwn01sb[:], in_=dwn01_d.ap())
            dwn2sb = wpool.tile([128, nblocks * _MH * 128], fp8)
            nc.sync.dma_start(out=dwn2sb[:], in_=dwn2_d.ap())
            cst = wpool.tile([128, nblocks * _NCC], f32)
            nc.sync.dma_start(out=cst[:], in_=cst_d.ap())

            # static S2 ring: pads memset once, interiors rewritten per use
            s2ring = wpool.tile([128, _NS2 * S2W], fp8)
            for r in range(_NS2):
                nc.gpsimd.memset(s2ring[:, r * S2W:r * S2W + _PAD], 0.0)
                nc.gpsimd.memset(
                    s2ring[:, r * S2W + _PAD + T:(r + 1) * S2W], 0.0)

            def s2buf(slot, mh):
                r = (slot * _MH + mh) % _NS2
                return s2ring[:, r * S2W:(r + 1) * S2W]

            def w1t(i, mh):
                o = (i * _MH + mh) * 256
                return _mk3(w1sb[:, o:o + 256], 128, 128)

            def w2t(i, mc, pair):
                o = (i * _KC * 2 + mc * 2 + pair) * 256
                return _mk3(w2sb[:, o:o + 256], 128, 128)

            def dwpt(i, mh):
                o = (i * _MH + mh) * 256
                return _mk3(dwpsb[:, o:o + 256], 128, 128)

            def dwn01t(i, mh):
                o = (i * _MH + mh) * 256
                return _mk3(dwn01sb[:, o:o + 256], 128, 128)

            def dwn2t(i, mh):
                o = (i * _MH + mh) * 128
                return dwn2sb[:, o:o + 128]

            def cc(i, j):
                return cst[:, i * _NCC + j:i * _NCC + j + 1]

            Rb = {}
            for b in range(bs):
                Rb[b] = []
                for kc in range(_KC):
                    rt = rpool.tile([128, T], f32, tag="R",
                                    name=f"R_b{b}_{kc}")
                    nc.sync.dma_start(
                        out=rt[:], in_=x_d.ap()[b, kc * 128:(kc + 1) * 128, :])
                    Rb[b].append(rt)

            state = {}

            def emitA_alloc(k, b, i):
                S1 = s1pool.tile([128, _KC * T], fp8, tag="S1",
                                 name=f"S1_b{b}_i{i}")
                R = Rb[b]
                for kc in range(_KC):
                    # S1 = (R >= t1) in {0,1}, whole row at once
                    if _S1_ENGINE == 'gpsimd':
                        nc.gpsimd.tensor_tensor(
                            out=S1[:, kc * T:(kc + 1) * T], in0=R[kc][:],
                            in1=_bcast(cc(i, kc), T), op=ALU.is_ge)
                    else:
                        nc.vector.tensor_scalar(
                            S1[:, kc * T:(kc + 1) * T], R[kc][:],
                            cc(i, kc), 0.0, op0=ALU.is_ge, op1=ALU.subtract)
                state[(b, i)] = (S1, k)

            def emitA_g(b, i, g):
                S1, slot = state[(b, i)]
                c0 = g * _QC
                for mh in range(_MH):
                    ps = psmm.tile([128, _QC], f32, tag="mm",
                                   name=f"psA_{b}_{i}_{mh}_{g}")
                    for nn in range(2):
                        cn = c0 + nn * 512
                        rhs = _mk3(S1[:, cn:cn + 512], T, 512)
                        nc.tensor.matmul(
                            ps[:, nn * 512:(nn + 1) * 512],
                            w1t(i, mh), rhs, start=True, stop=True,
                            perf_mode=DRM)
                    # S2n = sign(tau2' - p1)
                    nc.scalar.activation(
                        s2buf(slot, mh)[:, _PAD + c0:_PAD + c0 + _QC],
                        ps[:], ACTF.Sign, bias=cc(i, 2 + mh), scale=-1.0)

            def emitBC_q(b, i, q):
                d = 2 ** i
                R = Rb[b]
                _, slot = state[(b, i)]
                c0 = q * _QC
                S3 = [s3pool.tile([128, 2 * _QC], fp8, tag="S3",
                                  name=f"S3_b{b}_i{i}_q{q}_p{p}")
                      for p in range(2)]
                for mh in range(_MH):
                    S2m = s2buf(slot, mh)
                    pd = psdw.tile([128, _QC], f32, tag="dw",
                                   name=f"psD_{b}_{i}_q{q}_{mh}")
                    s3out = S3[mh // 2][:, (mh % 2) * _QC:(mh % 2 + 1) * _QC]
                    if _path(i, mh) == 'S':
                        for nn in range(2):
                            w0 = _PAD + c0 + nn * 512
                            rhs = _mk3(S2m[:, w0 - d:w0 - d + 512],
                                       2 * d, 512)
                            nc.tensor.matmul(
                                pd[:, nn * 512:(nn + 1) * 512],
                                dwpt(i, mh), rhs, start=True, stop=True,
                                perf_mode=DRM)
                        # S3 = ((pd - tau3) >= -s_c) = (q >= tau3) in {0,1}
                        nc.vector.scalar_tensor_tensor(
                            s3out, pd[:], cc(i, 6 + mh),
                            S2m[:, _PAD + c0:_PAD + c0 + _QC],
                            op0=ALU.subtract, op1=ALU.is_ge)
                    else:
                        for nn in range(2):
                            w0 = _PAD + c0 + nn * 512
                            rhs01 = _mk3(S2m[:, w0 - d:w0 - d + 512],
                                         d, 512)
                            nc.tensor.matmul(
                                pd[:, nn * 512:(nn + 1) * 512],
                                dwn01t(i, mh), rhs01,
                                start=True, stop=False, perf_mode=DRM)
                            nc.tensor.matmul(
                                pd[:, nn * 512:(nn + 1) * 512],
                                dwn2t(i, mh),
                                S2m[:, w0 + d:w0 + d + 512],
                                start=False, stop=True)
                        # S3 = sign(q - tau3) in {-1, +1}
                        nc.scalar.sign(s3out, pd[:], bias=cc(i, 6 + mh))
                for mc in range(_KC):
                    ps2 = psmm.tile([128, _QC], f32, tag="mm",
                                    name=f"psC_{b}_{i}_q{q}_{mc}")
                    for nn in range(2):
                        for pair in range(2):
                            rhs = _mk3(S3[pair][:, nn * 512:nn * 512 + 512],
                                       _QC, 512)
                            nc.tensor.matmul(
                                ps2[:, nn * 512:(nn + 1) * 512],
                                w2t(i, mc, pair), rhs,
                                start=(pair == 0), stop=(pair == 1),
                                perf_mode=DRM)
                    # R = (p2 - corr) + R   (corr integer => bit-exact)
                    nc.vector.scalar_tensor_tensor(
                        R[mc][:, c0:c0 + _QC], ps2[:], cc(i, 10 + mc),
                        R[mc][:, c0:c0 + _QC],
                        op0=ALU.subtract, op1=ALU.add)

            # software-pipelined emission: stage A of step k+1 interleaves
            # chunk-wise with stage B/C of step k so every engine streams
            seq = [(b, i) for i in range(nblocks) for b in range(bs)]
            emitA_alloc(0, *seq[0])
            for g in range(nq):
                emitA_g(*seq[0], g)
            for k in range(len(seq)):
                if k + 1 < len(seq):
                    emitA_alloc(k + 1, *seq[k + 1])
                for g in range(nq):
                    emitBC_q(*seq[k], g)
                    if k + 1 < len(seq):
                        emitA_g(*seq[k + 1], g)
                state.pop(seq[k])

            for b in range(bs):
                for kc in range(_KC):
                    nc.sync.dma_start(
                        out=out_d.ap()[b, kc * 128:(kc + 1) * 128, :],
                        in_=Rb[b][kc][:])
    nc.finalize()
    return nc


def _prep(inputs, nblocks=_BLOCKS):
    """Host-side weight/threshold preprocessing (tiny tensors only)."""
    e4 = ml_dtypes.float8_e4m3

    def thr(g, bb, m, v):
        return (m - bb * np.sqrt(v + _EPS) / g).astype(np.float32)

    w1dr = np.zeros((128, nblocks * _MH * 256), np.float32)
    w2dr = np.zeros((128, nblocks * _KC * 2 * 256), np.float32)
    dwp = np.zeros((128, nblocks * _MH * 256), np.float32)
    dwn01 = np.zeros((128, nblocks * _MH * 256), np.float32)
    dwn2 = np.zeros((128, nblocks * _MH * 128), np.float32)
    cst = np.zeros((128, nblocks * _NCC), np.float32)
    ar = np.arange(128)
    for i in range(nblocks):
        t1 = thr(inputs['bn1_gamma'][i], inputs['bn1_beta'][i],
                 inputs['bn1_mean'][i], inputs['bn1_var'][i])          # [Cb]
        t2 = thr(inputs['bn2_gamma'][i], inputs['bn2_beta'][i],
                 inputs['bn2_mean'][i], inputs['bn2_var'][i])          # [H]
        t3 = thr(inputs['bn3_gamma'][i], inputs['bn3_beta'][i],
                 inputs['bn3_mean'][i], inputs['bn3_var'][i])          # [H]
        W1s = np.sign(inputs['w1'][i]).astype(np.float32)              # [H, Cb]
        W2s = np.sign(inputs['w2'][i]).astype(np.float32)              # [Cb, H]
        dws = np.sign(inputs['dw_w'][i]).astype(np.float32)            # [H, 3]
        ctr = dws[:, 1]
        a0 = dws[:, 0] * ctr
        a2 = dws[:, 2] * ctr
        tau3 = ctr * t3                                                # [H]
        # S1 is {0,1}: p1' = W1s@b1, true p1 = 2*p1' - rowsum(W1s):
        # sign(p1-t2) = sign(p1' - tau2'), tau2' = (rowsum+t2)/2
        rs1 = W1s.sum(axis=1)                                          # [H]
        tau2p = 0.5 * (rs1 + t2)
        # conv2 weights and corr: 'S' columns {0,1} -> weight 2*W2s*ctr,
        # corr += rowsum of W2s*ctr; 'A' columns stay +-1
        w2eff = W2s * ctr[None, :]                                     # [Cb, H]
        amp = np.array([2.0 if _path(i, mh) == 'S' else 1.0
                        for mh in range(_MH)])
        W2use = w2eff * np.repeat(amp, 128)[None, :]
        sel = np.repeat(np.array([_path(i, mh) == 'S'
                                  for mh in range(_MH)]), 128)
        corr = (w2eff * sel[None, :]).sum(axis=1)                      # [Cb]
        for mh in range(_MH):
            o = (i * _MH + mh) * 256
            for j in range(2):
                w1dr[:, o + j * 128:o + (j + 1) * 128] = \
                    W1s[mh * 128:(mh + 1) * 128, j * 128:(j + 1) * 128].T
        for mc in range(_KC):
            for pair in range(2):
                o = (i * _KC * 2 + mc * 2 + pair) * 256
                for j in range(2):
                    kh = pair * 2 + j
                    w2dr[:, o + j * 128:o + (j + 1) * 128] = \
                        W2use[mc * 128:(mc + 1) * 128,
                              kh * 128:(kh + 1) * 128].T
        for mh in range(_MH):
            sl = slice(mh * 128, (mh + 1) * 128)
            o = (i * _MH + mh) * 256
            o2 = (i * _MH + mh) * 128
            # 'S': single-pass dw on S2n (= -s): j0 diag(-a0) tap -d,
            # j1 diag(-a2) tap +d
            dwp[ar, o + ar] = -a0[sl]
            dwp[ar, o + 128 + ar] = -a2[sl]
            # 'A': 3-tap dw on S2n: j0 diag(-a0) tap -d, j1 diag(-1)
            # tap 0, + diag(-a2) tap +d
            dwn01[ar, o + ar] = -a0[sl]
            dwn01[ar, o + 128 + ar] = -1.0
            dwn2[ar, o2 + ar] = -a2[sl]
        base = i * _NCC
        for kc in range(_KC):
            cst[:, base + kc] = t1[kc * 128:(kc + 1) * 128]
        for mh in range(_MH):
            sl = slice(mh * 128, (mh + 1) * 128)
            cst[:, base + 2 + mh] = tau2p[sl]
            cst[:, base + 6 + mh] = (tau3[sl] if _path(i, mh) == 'S'
                                     else -tau3[sl])
        for mc in range(_KC):
            cst[:, base + 10 + mc] = corr[mc * 128:(mc + 1) * 128]
    return (w1dr.astype(e4), w2dr.astype(e4), dwp.astype(e4),
            dwn01.astype(e4), dwn2.astype(e4), cst)


def kernel(**inputs):
    inputs = {k: np.asarray(v) for k, v in inputs.items()}
    x = inputs['x'].astype(np.float32)
    w1dr, w2dr, dwp, dwn01, dwn2, cst = _prep(inputs)

    if 'nc' not in _nc_cache:
        _nc_cache['nc'] = _build_nc()
    nc = _nc_cache['nc']

    in_maps = []
    for c in range(_NCORES):
        in_maps.append({
            'x': np.ascontiguousarray(x[c * _BS:(c + 1) * _BS]),
            'w1dr': w1dr, 'w2dr': w2dr, 'dwp': dwp, 'dwn01': dwn01,
            'dwn2': dwn2, 'cst': cst,
        })

    from concourse.bass_utils import run_bass_kernel_spmd
    import os
    trace = bool(int(os.environ.get('KERNEL_TRACE', '0')))
    res = run_bass_kernel_spmd(nc, in_maps, core_ids=list(range(_NCORES)),
                               trace=trace)
    _nc_cache['last_result'] = res
    out = np.concatenate([r['out'] for r in res.results], axis=0)
    return out.astype(np.float32)


# revision 6
# speedup vs baseline: 3.5108x; 1.0144x over previous
"""Bass/Trainium2 kernel for nn_BitwiseTasNetRepeat.

Strategy (v5: single-pass dw + STT fusion + 4-engine balance)
-------------------------------------------------------------
Each block collapses to threshold arithmetic on binary values:

    S1 = (R >= t1)                      {0,1} fp8  (GpSimd tensor_tensor
                                         vs stride-0 broadcast of t1)
    p1 = W1s @ S1                       (TensorE fp8 DR, K=256; rowsum of
                                         W1s folded into tau2')
    S2n = sign(tau2' - p1)              = -s, +-1 fp8 (ACT, scale=-1)
    per (i, mh) 'S' slots (~2/3):
      pd = a0*s(-d) + a2*s(+d)          (ONE DR matmul: taps -d,+d via
                                         pair-stride 2d, diag -a0/-a2)
      S3 = ((pd - tau3) >= S2n(0))      = (q >= tau3) {0,1}
                                        (DVE scalar_tensor_tensor: center
                                         tap fused into the compare)
    per (i, mh) 'A' slots (~1/3, offloads DVE -> ACT/TensorE):
      pd = full 3-tap q                 (2 matmuls as in v3)
      S3 = sign(q - tau3)               +-1  (ACT sign)
    p2 = W2' @ S3                       (TensorE fp8 DR, K=512; 'S' cols
                                         2*W2*ctr, 'A' cols W2*ctr)
    R  = (p2 - corr) + R                (DVE STT; corr = integer rowsum
                                         of W2*ctr over 'S' slots -- adds
                                         bit-exactly, preserving the
                                         reference's fp32 R arithmetic)

S2 lives in a statically-allocated ring so the halo pads are memset once
at startup instead of every step.  Data-parallel over batch, 2 samples
per core, 8 cores.  Stage A of step k+1 interleaves chunk-wise with
stage B/C of step k so every engine streams.
"""

import numpy as np
import ml_dtypes

_B, _CB, _H, _T = 16, 256, 512, 4096
_BLOCKS = 8
_EPS = 1e-5
_NCORES = 8
_BS = _B // _NCORES      # batches per core
_KC = _CB // 128         # 2  k-tiles of Cb
_MH = _H // 128          # 4  m-tiles of H
_PAD = 128               # halo for dilated depthwise conv (max d = 128)
_NCC = 12                # f32 const columns per block
_QC = 1024               # chunk width for dw/S3/conv2 stages
_NS2 = 10                # S2 ring buffers

_S1_ENGINE = 'vector'    # 'gpsimd' | 'vector'  (gpsimd is_ge: walrus crash)
_MH_ORDER = (0, 2, 1, 3)  # alternate A/S so ACT+DVE consumers interleave


def _path(i, mh):
    # 'A' = 3-tap dw + ACT sign S3; 'S' = single-MM dw + DVE STT S3
    return 'A' if mh < 2 else 'S'


_nc_cache = {}


def _mk3(ap2d, j_step, cols):
    """3D AP [128, 2 (stride j_step), cols] over a 2D row view."""
    import bass_rust
    v = ap2d.copy()
    l = v.ap
    v.ap = bass_rust.VecI64Pair([list(l[0]), [j_step, 2], [1, cols]])
    return v


def _bcast(ap2d, cols):
    """[128, 1] AP -> [128, cols] broadcast AP (free stride 0)."""
    import bass_rust
    v = ap2d.copy()
    l = v.ap
    v.ap = bass_rust.VecI64Pair([list(l[0]), [0, cols]])
    return v


def _build_nc(bs=_BS, nblocks=_BLOCKS, T=_T):
    import concourse.mybir as mybir
    from concourse import bacc
    from concourse.tile import TileContext

    f32 = mybir.dt.float32
    fp8 = mybir.dt.float8e4
    ALU = mybir.AluOpType
    ACTF = mybir.ActivationFunctionType
    DRM = mybir.MatmulPerfMode.DoubleRow
    nq = T // _QC
    S2W = T + 2 * _PAD

    nc = bacc.Bacc("TRN2", target_bir_lowering=False, debug=False,
                   enable_asserts=False)

    x_d = nc.dram_tensor("x", [bs, _CB, T], f32, kind="ExternalInput")
    w1_d = nc.dram_tensor("w1dr", [128, nblocks * _MH * 256], fp8,
                          kind="ExternalInput")
    w2_d = nc.dram_tensor("w2dr", [128, nblocks * _KC * 2 * 256], fp8,
                          kind="ExternalInput")
    dwp_d = nc.dram_tensor("dwp", [128, nblocks * _MH * 256], fp8,
                           kind="ExternalInput")
    dwn01_d = nc.dram_tensor("dwn01", [128, nblocks * _MH * 256], fp8,
                             kind="ExternalInput")
    dwn2_d = nc.dram_tensor("dwn2", [128, nblocks * _MH * 128], fp8,
                            kind="ExternalInput")
    cst_d = nc.dram_tensor("cst", [128, nblocks * _NCC], f32,
                           kind="ExternalInput")
    out_d = nc.dram_tensor("out", [bs, _CB, T], f32, kind="ExternalOutput")

    with TileContext(nc) as tc:
        with (
            tc.tile_pool(name="wpool", bufs=1) as wpool,
            tc.tile_pool(name="rpool", bufs=4) as rpool,
            tc.tile_pool(name="s1pool", bufs=4) as s1pool,
            tc.tile_pool(name="s3pool", bufs=8) as s3pool,
            tc.tile_pool(name="psmm", bufs=2, space="PSUM") as psmm,
            tc.tile_pool(name="psdw", bufs=2, space="PSUM") as psdw,
        ):
            w1sb = wpool.tile([128, nblocks * _MH * 256], fp8)
            nc.sync.dma_start(out=w1sb[:], in_=w1_d.ap())
            w2sb = wpool.tile([128, nblocks * _KC * 2 * 256], fp8)
            nc.sync.dma_start(out=w2sb[:], in_=w2_d.ap())
            dwpsb = wpool.tile([128, nblocks * _MH * 256], fp8)
            nc.sync.dma_start(out=dwpsb[:], in_=dwp_d.ap())
            dwn01sb = wpool.tile([128, nblocks * _MH * 256], fp8)
            nc.sync.dma_start(out=dwn01sb[:], in_=dwn01_d.ap())
            dwn2sb = wpool.tile([128, nblocks * _MH * 128], fp8)
            nc.sync.dma_start(out=dwn2sb[:], in_=dwn2_d.ap())
            cst = wpool.tile([128, nblocks * _NCC], f32)
            nc.sync.dma_start(out=cst[:], in_=cst_d.ap())

            # static S2 ring: pads memset once, interiors rewritten per use
            s2ring = wpool.tile([128, _NS2 * S2W], fp8)
            for r in range(_NS2):
                nc.gpsimd.memset(s2ring[:, r * S2W:r * S2W + _PAD], 0.0)
                nc.gpsimd.memset(
                    s2ring[:, r * S2W + _PAD + T:(r + 1) * S2W], 0.0)

            def s2buf(slot, mh):
                r = (slot * _MH + mh) % _NS2
                return s2ring[:, r * S2W:(r + 1) * S2W]

            def w1t(i, mh):
                o = (i * _MH + mh) * 256
                return _mk3(w1sb[:, o:o + 256], 128, 128)

            def w2t(i, mc, pair):
                o = (i * _KC * 2 + mc * 2 + pair) * 256
                return _mk3(w2sb[:, o:o + 256], 128, 128)

            def dwpt(i, mh):
                o = (i * _MH + mh) * 256
                return _mk3(dwpsb[:, o:o + 256], 128, 128)

            def dwn01t(i, mh):
                o = (i * _MH + mh) * 256
                return _mk3(dwn01sb[:, o:o + 256], 128, 128)

            def dwn2t(i, mh):
                o = (i * _MH + mh) * 128
                return dwn2sb[:, o:o + 128]

            def cc(i, j):
                return cst[:, i * _NCC + j:i * _NCC + j + 1]

            Rb = {}
            for b in range(bs):
                Rb[b] = []
                for kc in range(_KC):
                    rt = rpool.tile([128, T], f32, tag="R",
                                    name=f"R_b{b}_{kc}")
                    nc.sync.dma_start(
                        out=rt[:], in_=x_d.ap()[b, kc * 128:(kc + 1) * 128, :])
                    Rb[b].append(rt)

            state = {}

            def emitA_alloc(k, b, i):
                S1 = s1pool.tile([128, _KC * T], fp8, tag="S1",
                                 name=f"S1_b{b}_i{i}")
                R = Rb[b]
                for kc in range(_KC):
                    # S1 = (R >= t1) in {0,1}, whole row at once
                    if _S1_ENGINE == 'gpsimd':
                        nc.gpsimd.tensor_tensor(
                            out=S1[:, kc * T:(kc + 1) * T], in0=R[kc][:],
                            in1=_bcast(cc(i, kc), T), op=ALU.is_ge)
                    else:
                        nc.vector.tensor_scalar(
                            S1[:, kc * T:(kc + 1) * T], R[kc][:],
                            cc(i, kc), 0.0, op0=ALU.is_ge, op1=ALU.subtract)
                state[(b, i)] = (S1, k)

            def emitA_g(b, i, g):
                S1, slot = state[(b, i)]
                c0 = g * _QC
                for mh in range(_MH):
                    ps = psmm.tile([128, _QC], f32, tag="mm",
                                   name=f"psA_{b}_{i}_{mh}_{g}")
                    for nn in range(2):
                        cn = c0 + nn * 512
                        rhs = _mk3(S1[:, cn:cn + 512], T, 512)
                        nc.tensor.matmul(
                            ps[:, nn * 512:(nn + 1) * 512],
                            w1t(i, mh), rhs, start=True, stop=True,
                            perf_mode=DRM)
                    # S2n = sign(tau2' - p1)
                    nc.scalar.activation(
                        s2buf(slot, mh)[:, _PAD + c0:_PAD + c0 + _QC],
                        ps[:], ACTF.Sign, bias=cc(i, 2 + mh), scale=-1.0)

            def emitBC_q(b, i, q):
                d = 2 ** i
                R = Rb[b]
                _, slot = state[(b, i)]
                c0 = q * _QC
                S3 = [s3pool.tile([128, 2 * _QC], fp8, tag="S3",
                                  name=f"S3_b{b}_i{i}_q{q}_p{p}")
                      for p in range(2)]
                for mh in _MH_ORDER:
                    S2m = s2buf(slot, mh)
                    pd = psdw.tile([128, _QC], f32, tag="dw",
                                   name=f"psD_{b}_{i}_q{q}_{mh}")
                    s3out = S3[mh // 2][:, (mh % 2) * _QC:(mh % 2 + 1) * _QC]
                    if _path(i, mh) == 'S':
                        for nn in range(2):
                            w0 = _PAD + c0 + nn * 512
                            rhs = _mk3(S2m[:, w0 - d:w0 - d + 512],
                                       2 * d, 512)
                            nc.tensor.matmul(
                                pd[:, nn * 512:(nn + 1) * 512],
                                dwpt(i, mh), rhs, start=True, stop=True,
                                perf_mode=DRM)
                        # S3 = ((pd - tau3) >= -s_c) = (q >= tau3) in {0,1}
                        nc.vector.scalar_tensor_tensor(
                            s3out, pd[:], cc(i, 6 + mh),
                            S2m[:, _PAD + c0:_PAD + c0 + _QC],
                            op0=ALU.subtract, op1=ALU.is_ge)
                    else:
                        for nn in range(2):
                            w0 = _PAD + c0 + nn * 512
                            rhs01 = _mk3(S2m[:, w0 - d:w0 - d + 512],
                                         d, 512)
                            nc.tensor.matmul(
                                pd[:, nn * 512:(nn + 1) * 512],
                                dwn01t(i, mh), rhs01,
                                start=True, stop=False, perf_mode=DRM)
                            nc.tensor.matmul(
                                pd[:, nn * 512:(nn + 1) * 512],
                                dwn2t(i, mh),
                                S2m[:, w0 + d:w0 + d + 512],
                                start=False, stop=True)
                        # S3 = sign(q - tau3) in {-1, +1}
                        nc.scalar.sign(s3out, pd[:], bias=cc(i, 6 + mh))
                for mc in range(_KC):
                    ps2 = psmm.tile([128, _QC], f32, tag="mm",
                                    name=f"psC_{b}_{i}_q{q}_{mc}")
                    for nn in range(2):
                        for pair in range(2):
                            rhs = _mk3(S3[pair][:, nn * 512:nn * 512 + 512],
                                       _QC, 512)
                            nc.tensor.matmul(
                                ps2[:, nn * 512:(nn + 1) * 512],
                                w2t(i, mc, pair), rhs,
                                start=(pair == 0), stop=(pair == 1),
                                perf_mode=DRM)
                    # R = (p2 - corr) + R   (corr integer => bit-exact)
                    nc.vector.scalar_tensor_tensor(
                        R[mc][:, c0:c0 + _QC], ps2[:], cc(i, 10 + mc),
                        R[mc][:, c0:c0 + _QC],
                        op0=ALU.subtract, op1=ALU.add)

            # software-pipelined emission: stage A of step k+1 interleaves
            # chunk-wise with stage B/C of step k so every engine streams
            seq = [(b, i) for i in range(nblocks) for b in range(bs)]
            emitA_alloc(0, *seq[0])
            for g in range(nq):
                emitA_g(*seq[0], g)
            for k in range(len(seq)):
                if k + 1 < len(seq):
                    emitA_alloc(k + 1, *seq[k + 1])
                for g in range(nq):
                    emitBC_q(*seq[k], g)
                    if k + 1 < len(seq):
                        emitA_g(*seq[k + 1], g)
                state.pop(seq[k])

            for b in range(bs):
                for kc in range(_KC):
                    nc.sync.dma_start(
                        out=out_d.ap()[b, kc * 128:(kc + 1) * 128, :],
                        in_=Rb[b][kc][:])
    nc.finalize()
    return nc


def _prep(inputs, nblocks=_BLOCKS):
    """Host-side weight/threshold preprocessing (tiny tensors only)."""
    e4 = ml_dtypes.float8_e4m3

    def thr(g, bb, m, v):
        return (m - bb * np.sqrt(v + _EPS) / g).astype(np.float32)

    w1dr = np.zeros((128, nblocks * _MH * 256), np.float32)
    w2dr = np.zeros((128, nblocks * _KC * 2 * 256), np.float32)
    dwp = np.zeros((128, nblocks * _MH * 256), np.float32)
    dwn01 = np.zeros((128, nblocks * _MH * 256), np.float32)
    dwn2 = np.zeros((128, nblocks * _MH * 128), np.float32)
    cst = np.zeros((128, nblocks * _NCC), np.float32)
    ar = np.arange(128)
    for i in range(nblocks):
        t1 = thr(inputs['bn1_gamma'][i], inputs['bn1_beta'][i],
                 inputs['bn1_mean'][i], inputs['bn1_var'][i])          # [Cb]
        t2 = thr(inputs['bn2_gamma'][i], inputs['bn2_beta'][i],
                 inputs['bn2_mean'][i], inputs['bn2_var'][i])          # [H]
        t3 = thr(inputs['bn3_gamma'][i], inputs['bn3_beta'][i],
                 inputs['bn3_mean'][i], inputs['bn3_var'][i])          # [H]
        W1s = np.sign(inputs['w1'][i]).astype(np.float32)              # [H, Cb]
        W2s = np.sign(inputs['w2'][i]).astype(np.float32)              # [Cb, H]
        dws = np.sign(inputs['dw_w'][i]).astype(np.float32)            # [H, 3]
        ctr = dws[:, 1]
        a0 = dws[:, 0] * ctr
        a2 = dws[:, 2] * ctr
        tau3 = ctr * t3                                                # [H]
        # S1 is {0,1}: p1' = W1s@b1, true p1 = 2*p1' - rowsum(W1s):
        # sign(p1-t2) = sign(p1' - tau2'), tau2' = (rowsum+t2)/2
        rs1 = W1s.sum(axis=1)                                          # [H]
        tau2p = 0.5 * (rs1 + t2)
        # conv2 weights and corr: 'S' columns {0,1} -> weight 2*W2s*ctr,
        # corr += rowsum of W2s*ctr; 'A' columns stay +-1
        w2eff = W2s * ctr[None, :]                                     # [Cb, H]
        amp = np.array([2.0 if _path(i, mh) == 'S' else 1.0
                        for mh in range(_MH)])
        W2use = w2eff * np.repeat(amp, 128)[None, :]
        sel = np.repeat(np.array([_path(i, mh) == 'S'
                                  for mh in range(_MH)]), 128)
        corr = (w2eff * sel[None, :]).sum(axis=1)                      # [Cb]
        for mh in range(_MH):
            o = (i * _MH + mh) * 256
            for j in range(2):
                w1dr[:, o + j * 128:o + (j + 1) * 128] = \
                    W1s[mh * 128:(mh + 1) * 128, j * 128:(j + 1) * 128].T
        for mc in range(_KC):
            for pair in range(2):
                o = (i * _KC * 2 + mc * 2 + pair) * 256
                for j in range(2):
                    kh = pair * 2 + j
                    w2dr[:, o + j * 128:o + (j + 1) * 128] = \
                        W2use[mc * 128:(mc + 1) * 128,
                              kh * 128:(kh + 1) * 128].T
        for mh in range(_MH):
            sl = slice(mh * 128, (mh + 1) * 128)
            o = (i * _MH + mh) * 256
            o2 = (i * _MH + mh) * 128
            # 'S': single-pass dw on S2n (= -s): j0 diag(-a0) tap -d,
            # j1 diag(-a2) tap +d
            dwp[ar, o + ar] = -a0[sl]
            dwp[ar, o + 128 + ar] = -a2[sl]
            # 'A': 3-tap dw on S2n: j0 diag(-a0) tap -d, j1 diag(-1)
            # tap 0, + diag(-a2) tap +d
            dwn01[ar, o + ar] = -a0[sl]
            dwn01[ar, o + 128 + ar] = -1.0
            dwn2[ar, o2 + ar] = -a2[sl]
        base = i * _NCC
        for kc in range(_KC):
            cst[:, base + kc] = t1[kc * 128:(kc + 1) * 128]
        for mh in range(_MH):
            sl = slice(mh * 128, (mh + 1) * 128)
            cst[:, base + 2 + mh] = tau2p[sl]
            cst[:, base + 6 + mh] = (tau3[sl] if _path(i, mh) == 'S'
                                     else -tau3[sl])
        for mc in range(_KC):
            cst[:, base + 10 + mc] = corr[mc * 128:(mc + 1) * 128]
    return (w1dr.astype(e4), w2dr.astype(e4), dwp.astype(e4),
            dwn01.astype(e4), dwn2.astype(e4), cst)


def kernel(**inputs):
    inputs = {k: np.asarray(v) for k, v in inputs.items()}
    x = inputs['x'].astype(np.float32)
    w1dr, w2dr, dwp, dwn01, dwn2, cst = _prep(inputs)

    if 'nc' not in _nc_cache:
        _nc_cache['nc'] = _build_nc()
    nc = _nc_cache['nc']

    in_maps = []
    for c in range(_NCORES):
        in_maps.append({
            'x': np.ascontiguousarray(x[c * _BS:(c + 1) * _BS]),
            'w1dr': w1dr, 'w2dr': w2dr, 'dwp': dwp, 'dwn01': dwn01,
            'dwn2': dwn2, 'cst': cst,
        })

    from concourse.bass_utils import run_bass_kernel_spmd
    import os
    trace = bool(int(os.environ.get('KERNEL_TRACE', '0')))
    res = run_bass_kernel_spmd(nc, in_maps, core_ids=list(range(_NCORES)),
                               trace=trace)
    _nc_cache['last_result'] = res
    out = np.concatenate([r['out'] for r in res.results], axis=0)
    return out.astype(np.float32)
